# revision 2
# baseline (speedup 1.0000x reference)
"""Trainium2 Bass kernel for nn_ExactAttention (B=2, N=2048, H=16, D=128, fp32).

Strategy (8 NeuronCores, batch*head parallel):
  - 32 (b,h) pairs sharded 4-per-core; host pre-transposes [B,N,H,D] -> [32,N,D],
    casts Q/K to fp16 (score err ~5e-3) and V to bf16.
  - Q_T/K_T [d, N] are built by xbar DMA-transpose during load (zero PE cost).
  - Per pair, per n-span of 1024: scores computed TRANSPOSED
    (scores_T[m_tile=128, n_span] = K_T.T @ Q_T, fp16 matmuls, fp32 PSUM).
  - exp is the roofline (Act engine: 1 elem/cyc/lane, ~1.14us per [128,1024]
    tile, 16 tiles/span): 13 tiles/span run exp(s - 87.995) on Act (bf16 out);
    3 tiles/span are offloaded to DVE via a one-pass Schraudolph trick:
    y16 = int16(max(s,0) * 128/ln2) interpreted as bf16 bits approximates
    exp(s)*2^-127 ~= exp(s - 87.995) to +-3%.  The Act bias -87.995 is chosen
    so both methods produce identically-scaled unnormalized weights; the
    shared constant cancels in the softmax normalization (done on host).
  - AV accumulates out_T[d, n_span] = sum_m V_chunk.T @ expT(bf16) in a single
    2-bank PSUM tile (both 512-halves per m-tile share one V weight load),
    staggered one m-tile behind QK so the PE never waits on the exp engines.
  - Z (softmax denominator): all-bf16 pairwise-tree adds on DVE (2x mode,
    533ns each); the [128, n_span] partial sums are DMA'd out raw and the
    final partition-axis sum, reciprocal, normalization and output transpose
    happen on the HOST (numpy) - no PE transposes / DVE reductions on device.
  - Outputs: o = unnormalized out_T (bf16) and z = Z partials (bf16).
"""
import sys

sys.path.insert(0, "/opt/trn_rl_repo")

import math

import ml_dtypes
import numpy as np

import concourse.bass as bass
import concourse.tile as tile
from concourse import bacc, mybir
from concourse.bass_utils import run_bass_kernel_spmd

F32 = mybir.dt.float32
F16 = mybir.dt.float16
BF16 = mybir.dt.bfloat16
I16 = mybir.dt.int16
AF = mybir.ActivationFunctionType
ALU = mybir.AluOpType

B, N, H, D = 2, 2048, 16, 128
P = 128
N_CORES = 8
PAIRS = B * H                  # 32
PAIRS_PER_CORE = PAIRS // N_CORES  # 4
M_TILES = N // P               # 16
SPAN = 1024                    # n-span processed per inner pipeline
SPANS = N // SPAN              # 2

# exp(s - EXP_SHIFT) is the shared weight scale for both exp paths; it
# cancels in the host-side normalization.  87.995 centers the Schraudolph
# sawtooth against the exact Act exp (127*ln2 = 88.0297 minus half the
# mean linear-interp deficit).
EXP_SHIFT = 87.995
A_SLOPE = 128.0 / math.log(2.0)     # 184.6650 - maps scores to bf16 bits
DVE_MTS = (5, 10, 15)               # m-tiles whose exp runs on DVE


def build_program(repeat=1):
    nc = bacc.Bacc("TRN2", target_bir_lowering=False, debug=False,
                   num_devices=N_CORES)

    qin = nc.dram_tensor("q", [PAIRS_PER_CORE, N, D], F16, kind="ExternalInput").ap()
    kin = nc.dram_tensor("k", [PAIRS_PER_CORE, N, D], F16, kind="ExternalInput").ap()
    vin = nc.dram_tensor("v", [PAIRS_PER_CORE, N, D], BF16, kind="ExternalInput").ap()
    out = nc.dram_tensor("o", [PAIRS_PER_CORE, SPANS, P, SPAN], BF16,
                         kind="ExternalOutput").ap()
    zout = nc.dram_tensor("z", [PAIRS_PER_CORE, SPANS, P, SPAN], BF16,
                          kind="ExternalOutput").ap()

    with tile.TileContext(nc) as tc:
        with (
            tc.tile_pool(name="const", bufs=1) as const_pool,
            tc.tile_pool(name="big", bufs=2) as big_pool,
            tc.tile_pool(name="expp", bufs=17) as exp_pool,
            tc.tile_pool(name="zp", bufs=2) as z_pool,
            tc.tile_pool(name="osb", bufs=2) as osb_pool,
            tc.tile_pool(name="ps_score", bufs=3, space="PSUM") as ps_score,
            tc.tile_pool(name="ps_out", bufs=1, space="PSUM") as ps_out,
        ):
            bias_c = const_pool.tile([P, 1], F32)
            nc.gpsimd.memset(bias_c[:], -EXP_SHIFT)

            def prep_pair(pi):
                """Load pair pi; Q_T/K_T [d, N] via xbar DMA-transpose, V natural."""
                vt = big_pool.tile([P, M_TILES, P], BF16, tag="vt")
                nc.sync.dma_start(
                    vt[:], vin[pi].rearrange("(t p) d -> p t d", p=P))
                kt = big_pool.tile([P, N], F16, tag="kt")
                qt = big_pool.tile([P, N], F16, tag="qt")
                Q4 = N // 4
                for h in range(4):  # quarters so span-0 QK starts sooner
                    nc.sync.dma_start_transpose(
                        kt[:, h * Q4:(h + 1) * Q4],
                        kin[pi, h * Q4:(h + 1) * Q4, :])
                    nc.sync.dma_start_transpose(
                        qt[:, h * Q4:(h + 1) * Q4],
                        qin[pi, h * Q4:(h + 1) * Q4, :])
                return qt, kt, vt

            def do_span(pi, s, qt, kt, vt):
                n0 = s * SPAN
                outp = ps_out.tile([P, SPAN], F32, tag="outp")
                ets = {}
                leaves = {}
                ups = {}

                def av(mt):
                    for c in range(2):
                        nc.tensor.matmul(
                            outp[:, c * 512:(c + 1) * 512],
                            vt[:, mt, :], ets[mt][:, c * 512:(c + 1) * 512],
                            start=(mt == 0), stop=(mt == M_TILES - 1))

                for mt in range(M_TILES):
                    sc = ps_score.tile([P, SPAN], F32, tag="score")
                    for c in range(2):
                        nc.tensor.matmul(
                            sc[:, c * 512:(c + 1) * 512],
                            kt[:, mt * P:(mt + 1) * P],
                            qt[:, n0 + c * 512: n0 + (c + 1) * 512],
                            start=True, stop=True)
                    et = exp_pool.tile([P, SPAN], BF16, tag="expt")
                    if mt in DVE_MTS:
                        with nc.allow_low_precision(reason="schraudolph exp bits"):
                            nc.vector.tensor_scalar(
                                et[:].bitcast(I16), sc[:], 0.0, A_SLOPE,
                                ALU.max, ALU.mult)
                    else:
                        nc.scalar.activation(et[:], sc[:], AF.Exp,
                                             bias=bias_c[:], scale=1.0)
                    ets[mt] = et
                    # AV for the previous m-tile: staggered so the PE stream
                    # never catches up with the exp engines.
                    if mt >= 1:
                        av(mt - 1)
                    # Z tree: all-bf16 pairwise adds on DVE (2x mode)
                    if mt % 2 == 1:
                        li = mt // 2
                        lt = z_pool.tile([P, SPAN], BF16, tag=f"zleaf{li % 4}")
                        with nc.allow_low_precision(reason="bf16 Z tree"):
                            nc.vector.tensor_add(lt[:], ets[mt - 1][:], ets[mt][:])
                            leaves[li] = lt
                            if li == 1:
                                ups["m0"] = z_pool.tile([P, SPAN], BF16, tag="zm0", name="zm0")
                                nc.vector.tensor_add(
                                    ups["m0"][:], leaves[0][:], leaves[1][:])
                            elif li == 3:
                                ups["m1"] = z_pool.tile([P, SPAN], BF16, tag="zm1", name="zm1")
                                nc.vector.tensor_add(
                                    ups["m1"][:], leaves[2][:], leaves[3][:])
                            elif li == 4:
                                ups["n0"] = z_pool.tile([P, SPAN], BF16, tag="zn0", name="zn0")
                                nc.vector.tensor_add(
                                    ups["n0"][:], ups["m0"][:], ups["m1"][:])
                            elif li == 5:
                                ups["m2"] = z_pool.tile([P, SPAN], BF16, tag="zm2", name="zm2")
                                nc.vector.tensor_add(
                                    ups["m2"][:], leaves[4][:], leaves[5][:])

                av(M_TILES - 1)

                # Z tail merges + partials out (host does the partition sum)
                with nc.allow_low_precision(reason="bf16 Z tree"):
                    m3 = z_pool.tile([P, SPAN], BF16, tag="zm3")
                    nc.vector.tensor_add(m3[:], leaves[6][:], leaves[7][:])
                    n1t = z_pool.tile([P, SPAN], BF16, tag="zn1")
                    nc.vector.tensor_add(n1t[:], ups["m2"][:], m3[:])
                    zs = z_pool.tile([P, SPAN], BF16, tag="zsum")
                    nc.vector.tensor_add(zs[:], ups["n0"][:], n1t[:])
                nc.sync.dma_start(zout[pi, s], zs[:])

                # unnormalized out_T psum -> sbuf bf16 -> DRAM
                osc = osb_pool.tile([P, SPAN], BF16, tag="osc")
                with nc.allow_low_precision(reason="bf16 unnormalized out"):
                    nc.vector.tensor_copy(osc[:], outp[:])
                nc.sync.dma_start(out[pi, s], osc[:])

            for _rep in range(repeat):
                for pi in range(PAIRS_PER_CORE):
                    qt, kt, vt = prep_pair(pi)
                    for s in range(SPANS):
                        do_span(pi, s, qt, kt, vt)

    nc.compile()
    return nc


_NC = None


def _get_nc():
    global _NC
    if _NC is None:
        _NC = build_program()
    return _NC


def _shard_inputs(query, key, value):
    """Host prep: [B,N,H,D] f32 -> per-core {q,k,v} slices in [pairs,N,D]."""
    bf = ml_dtypes.bfloat16
    q = np.ascontiguousarray(np.asarray(query, np.float32)
                             .transpose(0, 2, 1, 3).reshape(PAIRS, N, D)).astype(np.float16)
    k = np.ascontiguousarray(np.asarray(key, np.float32)
                             .transpose(0, 2, 1, 3).reshape(PAIRS, N, D)).astype(np.float16)
    v = np.ascontiguousarray(np.asarray(value, np.float32)
                             .transpose(0, 2, 1, 3).reshape(PAIRS, N, D)).astype(bf)
    ppc = PAIRS_PER_CORE
    return [
        {"q": q[c * ppc:(c + 1) * ppc],
         "k": k[c * ppc:(c + 1) * ppc],
         "v": v[c * ppc:(c + 1) * ppc]}
        for c in range(N_CORES)
    ]


def _gather_output(res):
    """Host post: normalize by Z, transpose out_T -> [B,H,N,D] f32."""
    o = np.concatenate([np.asarray(res.results[c]["o"]) for c in range(N_CORES)],
                       axis=0).astype(np.float32)          # [32, SPANS, P, SPAN]
    z = np.concatenate([np.asarray(res.results[c]["z"]) for c in range(N_CORES)],
                       axis=0).astype(np.float32)          # [32, SPANS, P, SPAN]
    Z = z.sum(axis=2)                                      # [32, SPANS, SPAN]
    o = o / Z[:, :, None, :]
    o = o.transpose(0, 2, 1, 3).reshape(PAIRS, D, N)       # [pair, d, n]
    o = o.transpose(0, 2, 1)                               # [pair, n, d]
    return np.ascontiguousarray(o).reshape(B, H, N, D)


def kernel(query: np.ndarray, key: np.ndarray, value: np.ndarray) -> np.ndarray:
    nc = _get_nc()
    in_maps = _shard_inputs(query, key, value)
    res = run_bass_kernel_spmd(nc, in_maps, list(range(N_CORES)), trace=False)
    return _gather_output(res)


# revision 6
# speedup vs baseline: 1.0047x; 1.0047x over previous
"""Trainium2 Bass kernel for nn_ExactAttention (B=2, N=2048, H=16, D=128, fp32).

Strategy (8 NeuronCores, batch*head parallel):
  - 32 (b,h) pairs sharded 4-per-core; host pre-transposes [B,N,H,D] -> [32,N,D],
    casts Q/K to fp16 (score err ~5e-3) and V to bf16.
  - Q_T/K_T [d, N] are built by xbar DMA-transpose during load (zero PE cost).
  - Per pair, per n-span of 1024: scores computed TRANSPOSED
    (scores_T[m_tile=128, n_span] = K_T.T @ Q_T, fp16 matmuls, fp32 PSUM).
  - exp is the roofline (Act engine: 1 elem/cyc/lane, ~1.14us per [128,1024]
    tile, 16 tiles/span): 13 tiles/span run exp(s - 87.995) on Act (bf16 out);
    3 tiles/span are offloaded to DVE via a one-pass Schraudolph trick:
    y16 = int16(max(s,0) * 128/ln2) interpreted as bf16 bits approximates
    exp(s)*2^-127 ~= exp(s - 87.995) to +-3%.  The Act bias -87.995 is chosen
    so both methods produce identically-scaled unnormalized weights; the
    shared constant cancels in the softmax normalization (done on host).
  - AV accumulates out_T[d, n_span] = sum_m V_chunk.T @ expT(bf16) in a single
    2-bank PSUM tile (both 512-halves per m-tile share one V weight load),
    staggered one m-tile behind QK so the PE never waits on the exp engines.
  - Z (softmax denominator): all-bf16 pairwise-tree adds on DVE (2x mode,
    533ns each); the [128, n_span] partial sums are DMA'd out raw and the
    final partition-axis sum, reciprocal, normalization and output transpose
    happen on the HOST (numpy) - no PE transposes / DVE reductions on device.
  - Outputs: o = unnormalized out_T (bf16) and z = Z partials (bf16).
"""
import sys

sys.path.insert(0, "/opt/trn_rl_repo")

import math

import ml_dtypes
import numpy as np

import concourse.bass as bass
import concourse.tile as tile
from concourse import bacc, mybir
from concourse.bass_utils import run_bass_kernel_spmd

F32 = mybir.dt.float32
F16 = mybir.dt.float16
BF16 = mybir.dt.bfloat16
I16 = mybir.dt.int16
AF = mybir.ActivationFunctionType
ALU = mybir.AluOpType

B, N, H, D = 2, 2048, 16, 128
P = 128
N_CORES = 8
PAIRS = B * H                  # 32
PAIRS_PER_CORE = PAIRS // N_CORES  # 4
M_TILES = N // P               # 16
SPAN = 1024                    # n-span processed per inner pipeline
SPANS = N // SPAN              # 2

# exp(s - EXP_SHIFT) is the shared weight scale for both exp paths; it
# cancels in the host-side normalization.  87.995 centers the Schraudolph
# sawtooth against the exact Act exp (127*ln2 = 88.0297 minus half the
# mean linear-interp deficit).
EXP_SHIFT = 87.995
A_SLOPE = 128.0 / math.log(2.0)     # 184.6650 - maps scores to bf16 bits
# m-tiles whose exp runs on DVE.  Slot 15 must stay on Act: a DVE slot-15
# sits behind the DVE tree queue and delays AV15 -> QK0(next span) on the
# in-order PE queue, stalling Act at every span boundary.
DVE_MTS = (4, 9, 14)


def build_program(repeat=1):
    nc = bacc.Bacc("TRN2", target_bir_lowering=False, debug=False,
                   num_devices=N_CORES)

    qin = nc.dram_tensor("q", [PAIRS_PER_CORE, N, D], F16, kind="ExternalInput").ap()
    kin = nc.dram_tensor("k", [PAIRS_PER_CORE, N, D], F16, kind="ExternalInput").ap()
    vin = nc.dram_tensor("v", [PAIRS_PER_CORE, N, D], BF16, kind="ExternalInput").ap()
    out = nc.dram_tensor("o", [PAIRS_PER_CORE, SPANS, P, SPAN], BF16,
                         kind="ExternalOutput").ap()
    zout = nc.dram_tensor("z", [PAIRS_PER_CORE, SPANS, P, SPAN], BF16,
                          kind="ExternalOutput").ap()

    with tile.TileContext(nc) as tc:
        with (
            tc.tile_pool(name="const", bufs=1) as const_pool,
            tc.tile_pool(name="big", bufs=2) as big_pool,
            tc.tile_pool(name="expp", bufs=17) as exp_pool,
            tc.tile_pool(name="zp", bufs=2) as z_pool,
            tc.tile_pool(name="osb", bufs=2) as osb_pool,
            tc.tile_pool(name="ps_score", bufs=3, space="PSUM") as ps_score,
            tc.tile_pool(name="ps_out", bufs=1, space="PSUM") as ps_out,
        ):
            bias_c = const_pool.tile([P, 1], F32)
            nc.gpsimd.memset(bias_c[:], -EXP_SHIFT)

            def prep_pair(pi):
                """Load pair pi; Q_T/K_T [d, N] via xbar DMA-transpose, V natural.

                Issue the 9 DMAs from 5 different engine queues (descriptor
                generation is ~0.7-1.2us per issue and serializes per queue),
                kt/qt quarter 0 first so span-0 QK starts as soon as possible.
                """
                vt = big_pool.tile([P, M_TILES, P], BF16, tag="vt")
                kt = big_pool.tile([P, N], F16, tag="kt")
                qt = big_pool.tile([P, N], F16, tag="qt")
                Q4 = N // 4
                # DMA-transpose issue is HWDGE-only (sync + scalar).  Scalar
                # is free during the prologue but is the bottleneck engine
                # mid-stream, so only pair 0 borrows it.
                engs = [nc.sync, nc.scalar] if pi == 0 else [nc.sync]
                issues = []
                for h in range(4):  # quarters so span-0 QK starts sooner
                    issues.append((kt[:, h * Q4:(h + 1) * Q4],
                                   kin[pi, h * Q4:(h + 1) * Q4, :]))
                    issues.append((qt[:, h * Q4:(h + 1) * Q4],
                                   qin[pi, h * Q4:(h + 1) * Q4, :]))
                for i, (dst, src) in enumerate(issues):
                    engs[i % len(engs)].dma_start_transpose(dst, src)
                nc.gpsimd.dma_start(
                    vt[:], vin[pi].rearrange("(t p) d -> p t d", p=P))
                return qt, kt, vt

            def do_span(pi, s, qt, kt, vt):
                n0 = s * SPAN
                outp = ps_out.tile([P, SPAN], F32, tag="outp")
                ets = {}
                leaves = {}
                ups = {}

                def av(mt):
                    for c in range(2):
                        nc.tensor.matmul(
                            outp[:, c * 512:(c + 1) * 512],
                            vt[:, mt, :], ets[mt][:, c * 512:(c + 1) * 512],
                            start=(mt == 0), stop=(mt == M_TILES - 1))

                for mt in range(M_TILES):
                    sc = ps_score.tile([P, SPAN], F32, tag="score")
                    for c in range(2):
                        nc.tensor.matmul(
                            sc[:, c * 512:(c + 1) * 512],
                            kt[:, mt * P:(mt + 1) * P],
                            qt[:, n0 + c * 512: n0 + (c + 1) * 512],
                            start=True, stop=True)
                    et = exp_pool.tile([P, SPAN], BF16, tag="expt")
                    if mt in DVE_MTS:
                        with nc.allow_low_precision(reason="schraudolph exp bits"):
                            nc.vector.tensor_scalar(
                                et[:].bitcast(I16), sc[:], 0.0, A_SLOPE,
                                ALU.max, ALU.mult)
                    else:
                        nc.scalar.activation(et[:], sc[:], AF.Exp,
                                             bias=bias_c[:], scale=1.0)
                    ets[mt] = et
                    # AV for the previous m-tile: staggered so the PE stream
                    # never catches up with the exp engines.
                    if mt >= 1:
                        av(mt - 1)
                    # Z tree: all-bf16 pairwise adds on DVE (2x mode)
                    if mt % 2 == 1:
                        li = mt // 2
                        lt = z_pool.tile([P, SPAN], BF16, tag=f"zleaf{li % 4}")
                        with nc.allow_low_precision(reason="bf16 Z tree"):
                            nc.vector.tensor_add(lt[:], ets[mt - 1][:], ets[mt][:])
                            leaves[li] = lt
                            if li == 1:
                                ups["m0"] = z_pool.tile([P, SPAN], BF16, tag="zm0", name="zm0")
                                nc.vector.tensor_add(
                                    ups["m0"][:], leaves[0][:], leaves[1][:])
                            elif li == 3:
                                ups["m1"] = z_pool.tile([P, SPAN], BF16, tag="zm1", name="zm1")
                                nc.vector.tensor_add(
                                    ups["m1"][:], leaves[2][:], leaves[3][:])
                            elif li == 4:
                                ups["n0"] = z_pool.tile([P, SPAN], BF16, tag="zn0", name="zn0")
                                nc.vector.tensor_add(
                                    ups["n0"][:], ups["m0"][:], ups["m1"][:])
                            elif li == 5:
                                ups["m2"] = z_pool.tile([P, SPAN], BF16, tag="zm2", name="zm2")
                                nc.vector.tensor_add(
                                    ups["m2"][:], leaves[4][:], leaves[5][:])

                av(M_TILES - 1)

                # unnormalized out_T psum -> sbuf bf16 -> DRAM.  Emitted
                # before the Z tail merges (and split in halves for subtile
                # overlap with AV15) so the outp PSUM frees promptly for the
                # next span's AV.
                osc = osb_pool.tile([P, SPAN], BF16, tag="osc")
                with nc.allow_low_precision(reason="bf16 unnormalized out"):
                    nc.vector.tensor_copy(osc[:, 0:512], outp[:, 0:512])
                    nc.vector.tensor_copy(osc[:, 512:1024], outp[:, 512:1024])
                nc.sync.dma_start(out[pi, s], osc[:])

                # Z tail merges + partials out (host does the partition sum)
                with nc.allow_low_precision(reason="bf16 Z tree"):
                    m3 = z_pool.tile([P, SPAN], BF16, tag="zm3")
                    nc.vector.tensor_add(m3[:], leaves[6][:], leaves[7][:])
                    n1t = z_pool.tile([P, SPAN], BF16, tag="zn1")
                    nc.vector.tensor_add(n1t[:], ups["m2"][:], m3[:])
                    zs = z_pool.tile([P, SPAN], BF16, tag="zsum")
                    nc.vector.tensor_add(zs[:], ups["n0"][:], n1t[:])
                nc.sync.dma_start(zout[pi, s], zs[:])

            for _rep in range(repeat):
                for pi in range(PAIRS_PER_CORE):
                    qt, kt, vt = prep_pair(pi)
                    for s in range(SPANS):
                        do_span(pi, s, qt, kt, vt)

    nc.compile()
    return nc


_NC = None


def _get_nc():
    global _NC
    if _NC is None:
        _NC = build_program()
    return _NC


def _shard_inputs(query, key, value):
    """Host prep: [B,N,H,D] f32 -> per-core {q,k,v} slices in [pairs,N,D]."""
    bf = ml_dtypes.bfloat16
    q = np.ascontiguousarray(np.asarray(query, np.float32)
                             .transpose(0, 2, 1, 3).reshape(PAIRS, N, D)).astype(np.float16)
    k = np.ascontiguousarray(np.asarray(key, np.float32)
                             .transpose(0, 2, 1, 3).reshape(PAIRS, N, D)).astype(np.float16)
    v = np.ascontiguousarray(np.asarray(value, np.float32)
                             .transpose(0, 2, 1, 3).reshape(PAIRS, N, D)).astype(bf)
    ppc = PAIRS_PER_CORE
    return [
        {"q": q[c * ppc:(c + 1) * ppc],
         "k": k[c * ppc:(c + 1) * ppc],
         "v": v[c * ppc:(c + 1) * ppc]}
        for c in range(N_CORES)
    ]


def _gather_output(res):
    """Host post: normalize by Z, transpose out_T -> [B,H,N,D] f32."""
    o = np.concatenate([np.asarray(res.results[c]["o"]) for c in range(N_CORES)],
                       axis=0).astype(np.float32)          # [32, SPANS, P, SPAN]
    z = np.concatenate([np.asarray(res.results[c]["z"]) for c in range(N_CORES)],
                       axis=0).astype(np.float32)          # [32, SPANS, P, SPAN]
    Z = z.sum(axis=2)                                      # [32, SPANS, SPAN]
    o = o / Z[:, :, None, :]
    o = o.transpose(0, 2, 1, 3).reshape(PAIRS, D, N)       # [pair, d, n]
    o = o.transpose(0, 2, 1)                               # [pair, n, d]
    return np.ascontiguousarray(o).reshape(B, H, N, D)


def kernel(query: np.ndarray, key: np.ndarray, value: np.ndarray) -> np.ndarray:
    nc = _get_nc()
    in_maps = _shard_inputs(query, key, value)
    res = run_bass_kernel_spmd(nc, in_maps, list(range(N_CORES)), trace=False)
    return _gather_output(res)
